# revision 31
# baseline (speedup 1.0000x reference)
"""OIM loss with circular queue — Trainium2 Bass kernel (8 NeuronCores).

Strategy (v7, fp8 DoubleRow end-to-end)
---------------------------------------
loss = mean_b [ M + log S_b - 30*cos(x_b, e_{xe_b}) ],
S_b = sum_{q good} exp(30*cos(x_b, e_q) - M), with e the post-update queue.

Device-side compute per core (tensor-parallel over Q):
  - one-hot label masks on DVE/Pool, per-pid means via fp8-DR matmuls
  - row norms of x: fp8-DR x@x^T per b-tile, diagonal gathered on DVE;
    1/sqrt via tensor_scalar max+pow
  - normalized means -> gated fp8 d-major tiles (PE transpose + cast)
  - big logits matmul in fp8e4 DoubleRow (K=256/pass) into [128,2048] PSUM
    (gated means accumulated onto cols 0:256 in the same group), exp on ACT
    (per-row scale=30*rin/SX, bias=-M), row-sums via a DVE tensor_scalar
    accumulate, target logits gathered from PSUM cols 0:256.

Layout trick: the 256 queue slots rewritten by the circular-queue update
("window") are core 0's first 256 columns (zeros in its eT8; the on-device
means land there through the extra matmul); the other 16128 original slots
fill the rest.  Bad slots (label IGNORE) are zero columns -> each adds
exactly exp(-M), subtracted on the host.  Every batch row's target is a
window slot, so its logit is read from PSUM cols 0:256 on core 0 (cores
1-7 compute garbage there which the host ignores).  Host does the integer
queue bookkeeping, fp8 quantization/transposes of inputs, and the final
log/mean; all O(B*D*Q) FLOPs run on device.
"""

import os
import sys

import numpy as np

for _p in ("/opt/trn_rl_repo", "/root/.axon_site/_ro/trn_rl_repo"):
    if os.path.isdir(_p) and _p not in sys.path:
        sys.path.insert(0, _p)

import ml_dtypes

B, D, Q, U = 4096, 512, 16384, 256
N_CORES = 8
QS = Q // N_CORES          # queue columns per core
W0 = U                     # window block size on core 0's layout
OIM_SCALAR = 30.0
IGNORE = -1
SXE = 16.0                 # fp8 scale for emb/mean operands
MT = B // 128              # 32 b-tiles
KP = 2                     # k-passes of 256 (DoubleRow)
KI = 2                     # interleave factor inside a pass
UT = U // 128              # 2 u-tiles
NQ = QS // 512             # 4 matmul n-chunks per core

# packed small-input column layout
_C_RCNT = 0
_C_GKR = _C_RCNT + UT
_C_WIDX = _C_GKR + UT
_C_IOTA = _C_WIDX + MT
_C_PIDX = _C_IOTA + U
_C_LABF = _C_PIDX + 1
SC = _C_LABF + MT

F8 = ml_dtypes.float8_e4m3
BF = ml_dtypes.bfloat16

_PROG_CACHE = {}


def _build_program(M: float):
    import concourse.bacc as bacc
    import concourse.tile as tile
    from concourse import mybir
    from concourse.masks import make_identity

    f32 = mybir.dt.float32
    bf16 = mybir.dt.bfloat16
    fp8 = mybir.dt.float8e4
    AF = mybir.ActivationFunctionType
    OP = mybir.AluOpType
    DR = mybir.MatmulPerfMode.DoubleRow

    nc = bacc.Bacc("TRN2", target_bir_lowering=False, debug=False,
                   num_devices=N_CORES)

    sm_d = nc.dram_tensor("smalls", [128, SC], f32, kind="ExternalInput").ap()
    unq_d = nc.dram_tensor("uniqb", [128, U], bf16, kind="ExternalInput").ap()
    xT8_d = nc.dram_tensor("xT8", [128, KP, KI, B], fp8, kind="ExternalInput").ap()
    x8b_d = nc.dram_tensor("x8b", [128, MT * D], fp8, kind="ExternalInput").ap()
    eT8_d = nc.dram_tensor("eT8", [128, KP, KI, QS], fp8, kind="ExternalInput").ap()
    osum_d = nc.dram_tensor("osum", [128, MT], f32, kind="ExternalOutput").ap()
    tco_d = nc.dram_tensor("tco", [128, MT], f32, kind="ExternalOutput").ap()

    with tile.TileContext(nc) as tc:
        with (
            tc.tile_pool(name="singles", bufs=1) as singles,
            tc.tile_pool(name="work", bufs=3) as work,
            tc.tile_pool(name="small", bufs=4) as small,
        ):
            ident = singles.tile([128, 128], bf16)
            make_identity(nc, ident)

            # one packed DMA for the small inputs, then big operands
            # interleaved so compute unblocks ASAP
            sm = singles.tile([128, SC], f32)
            nc.sync.dma_start(out=sm, in_=sm_d)
            uniqb = singles.tile([128, U], bf16)
            nc.sync.dma_start(out=uniqb, in_=unq_d)
            rcnt = sm[:, _C_RCNT:_C_RCNT + UT]
            gkr = sm[:, _C_GKR:_C_GKR + UT]
            widx = sm[:, _C_WIDX:_C_WIDX + MT]
            iotab = sm[:, _C_IOTA:_C_IOTA + U]
            pidx = sm[:, _C_PIDX:_C_PIDX + 1]
            labf = sm[:, _C_LABF:_C_LABF + MT]

            BC = B // 4   # b-range per xT8/x8b DMA chunk
            xT8 = singles.tile([128, KP, KI, B], fp8)
            x8b = singles.tile([128, MT, D], fp8)
            mk8 = singles.tile([128, MT, U], fp8)
            eT8 = singles.tile([128, KP, KI, QS], fp8)

            def xT8_chunk(h):
                nc.sync.dma_start(out=xT8[:, :, :, h * BC:(h + 1) * BC],
                                  in_=xT8_d[:, :, :, h * BC:(h + 1) * BC])

            def x8b_chunk(h):
                s = MT // 4 * h
                nc.sync.dma_start(out=x8b[:, s:s + MT // 4, :],
                                  in_=x8b_d[:, s * D:(s + MT // 4) * D])

            for h in range(4):
                xT8_chunk(h)
                x8b_chunk(h)
            for n in range(NQ):
                nc.sync.dma_start(out=eT8[:, :, :, n * 512:(n + 1) * 512],
                                  in_=eT8_d[:, :, :, n * 512:(n + 1) * 512])

            biasM = singles.tile([128, 1], f32)
            nc.vector.memset(biasM, -M)
            epsb = singles.tile([128, 1], f32)
            nc.vector.memset(epsb, 1e-24)
            # dummy sqrt: loads the sqrt table (sqrt+square) at t~0, off the
            # critical path; a dummy exp later swaps to the exp table before
            # phase C needs it
            ones = singles.tile([128, 1], f32)
            nc.vector.memset(ones, 1.0)
            tinya = singles.tile([128, 1], f32)
            nc.scalar.activation(out=tinya, in_=ones, func=AF.Sqrt)

            # one-hot masks mk8[:, m, u] = (labels[128m+p] == uniq[u]):
            # early tiles on DVE (idle until the diag loop), rest on Pool
            def emit_mask(m, eng):
                eng.tensor_scalar(out=mk8[:, m, :], in0=uniqb,
                                  scalar1=labf[:, m:m + 1], scalar2=None,
                                  op0=OP.is_equal)
            for m in range(12):
                emit_mask(m, nc.vector)
            for m in range(12, MT):
                emit_mask(m, nc.gpsimd)

            uembT8 = singles.tile([128, KP, KI, U], fp8)   # gated fp8 means^T
            nsq = singles.tile([128, MT], f32)             # |x_b|^2
            rin30 = singles.tile([128, MT], f32)           # 30/(SXE*|x_b|)
            macc = singles.tile([128, UT], f32)            # (sum/cnt)^2 norms
            osum = singles.tile([128, MT], f32)            # sum-exp collector
            tco = singles.tile([128, MT], f32)             # target-cos collector

            # ---------- phase A: masked means + row norms, chunk-wise ------
            with (
                tc.tile_pool(name="psum_u", bufs=1, space="PSUM") as psum_u,
                tc.tile_pool(name="psum_n", bufs=4, space="PSUM") as psum_n,
            ):
                ps_u = [psum_u.tile([128, D], f32, tag=f"uniq{mu}",
                                    name=f"ps_u{mu}") for mu in range(UT)]

                def means_pair(t):
                    for mu in range(UT):
                        nc.tensor.matmul(
                            ps_u[mu],
                            mk8[:, 2 * t:2 * t + 2, mu * 128:(mu + 1) * 128],
                            x8b[:, 2 * t:2 * t + 2, :],
                            start=(t == 0), stop=(t == MT // 2 - 1),
                            perf_mode=DR)

                def diag_tile(m):
                    psn = psum_n.tile([128, 512], f32, tag="psn")
                    dv = psn[:, 0:128]
                    xs = xT8[:, :, :, m * 128:(m + 1) * 128]
                    for kp in range(KP):
                        nc.tensor.matmul(dv, xs[:, kp, :, :], xs[:, kp, :, :],
                                         start=(kp == 0), stop=(kp == KP - 1),
                                         perf_mode=DR)
                    scr = work.tile([128, 128], f32, tag="dscr")
                    nc.vector.scalar_tensor_tensor(
                        out=scr, in0=iotab[:, 0:128], scalar=pidx,
                        in1=dv, op0=OP.is_equal, op1=OP.mult,
                        accum_out=nsq[:, m:m + 1])

                def norm_fin(ch):
                    # rin30 cols for this chunk: recip(sqrt(nsq*(SXE/30)^2))
                    sl = slice(8 * ch, 8 * ch + 8)
                    nrm = small.tile([128, 8], f32, tag="nrm")
                    nc.scalar.activation(out=nrm, in_=nsq[:, sl],
                                         func=AF.Sqrt, bias=epsb,
                                         scale=(SXE / OIM_SCALAR) ** 2)
                    nc.vector.reciprocal(rin30[:, sl], nrm)

                for ch in range(4):
                    for t in range(4 * ch, 4 * ch + 4):
                        means_pair(t)
                    for m in range(8 * ch, 8 * ch + 8):
                        diag_tile(m)
                    norm_fin(ch)

                # mean chain: squared mean norms straight from PSUM (ACT),
                # rmg = SXE*gate*ukeep*rcnt/|mean|
                for mu in range(UT):
                    sq2 = work.tile([128, D], bf16, tag="sq")
                    nc.scalar.activation(out=sq2, in_=ps_u[mu],
                                         func=AF.Square,
                                         scale=rcnt[:, mu:mu + 1],
                                         accum_out=macc[:, mu:mu + 1])
                mnr = small.tile([128, UT], f32, tag="mnr")
                nc.scalar.activation(out=mnr, in_=macc, func=AF.Sqrt,
                                     bias=epsb)
                # swap ACT to the exp table: all sqrt-table users are behind
                # us, phase C's exps ahead — the load runs in ACT's idle gap
                tinyb = small.tile([128, 1], f32, tag="tinyb")
                nc.scalar.activation(out=tinyb, in_=ones, func=AF.Exp)

                mrc = small.tile([128, UT], f32, tag="mrc")
                nc.vector.reciprocal(mrc, mnr)
                rmg = small.tile([128, UT], f32, tag="rmg")
                nc.vector.tensor_tensor(out=rmg, in0=mrc, in1=gkr,
                                        op=OP.mult)
                # normalized gated means (DVE) -> transpose (PE) -> fp8
                mng_t = [singles.tile([128, D], bf16, name=f"mng{mu}")
                         for mu in range(UT)]
                for mu in range(UT):
                    nc.vector.tensor_scalar_mul(out=mng_t[mu], in0=ps_u[mu],
                                                scalar1=rmg[:, mu:mu + 1])
                for mu in range(UT):
                    pst = psum_n.tile([128, D], bf16, tag="pst", bufs=2)
                    for kd in range(D // 128):
                        nc.tensor.transpose(pst[:, kd * 128:(kd + 1) * 128],
                                            mng_t[mu][:, kd * 128:(kd + 1) * 128],
                                            ident)
                    dst = uembT8[:, :, :, mu * 128:(mu + 1) * 128]
                    if mu == 0:
                        nc.scalar.activation(out=dst, in_=pst, func=AF.Copy)
                    else:
                        nc.vector.tensor_scalar_mul(out=dst, in0=pst,
                                                    scalar1=1.0)

            # ---------- phase C: logits + exp + sums + target gather -------
            with tc.tile_pool(name="psum_l", bufs=2, space="PSUM") as psum_l:
                for m in range(MT):
                    pl = psum_l.tile([128, NQ * 512], f32, tag="pl")
                    xs = xT8[:, :, :, m * 128:(m + 1) * 128]
                    for n in (1, 2, 3, 0):
                        for kp in range(KP):
                            nc.tensor.matmul(
                                pl[:, n * 512:(n + 1) * 512],
                                xs[:, kp, :, :],
                                eT8[:, kp, :, n * 512:(n + 1) * 512],
                                start=(kp == 0),
                                stop=(kp == KP - 1 and n != 0),
                                perf_mode=DR)
                    # gated means accumulate onto cols 0:U, same group as the
                    # n=0 chunk (eT8 cols 0:U are zero on core 0; uembT8 is
                    # zero on cores 1-7)
                    for kp in range(KP):
                        nc.tensor.matmul(
                            pl[:, 0:U], xs[:, kp, :, :], uembT8[:, kp, :, :],
                            start=False, stop=(kp == KP - 1),
                            perf_mode=DR, skip_group_check=True)
                    # target logit: window cols live at 0:U (core 0 layout)
                    scr = work.tile([128, U], f32, tag="scr")
                    nc.vector.scalar_tensor_tensor(
                        out=scr, in0=iotab, scalar=widx[:, m:m + 1],
                        in1=pl[:, 0:U], op0=OP.is_equal, op1=OP.mult,
                        accum_out=tco[:, m:m + 1])
                    ex = work.tile([128, NQ * 512], bf16, tag="ex", bufs=2)
                    nc.scalar.activation(out=ex, in_=pl, func=AF.Exp,
                                         bias=biasM, scale=rin30[:, m:m + 1])
                    nc.vector.tensor_scalar(out=ex, in0=ex, scalar1=1.0,
                                            scalar2=0.0, op0=OP.mult,
                                            op1=OP.add,
                                            accum_out=osum[:, m:m + 1])
                    if m == MT // 2 - 1:
                        nc.sync.dma_start(out=osum_d[:, 0:MT // 2],
                                          in_=osum[:, 0:MT // 2])
                nc.vector.tensor_tensor(out=tco, in0=tco, in1=rin30,
                                        op=OP.mult)

            nc.sync.dma_start(out=osum_d[:, MT // 2:], in_=osum[:, MT // 2:])
            nc.sync.dma_start(out=tco_d, in_=tco)

    nc.compile()
    return nc


def _host_bookkeeping(labels, label_cq, header_cq):
    """Mirror the reference's integer-only queue-update semantics."""
    labels = np.asarray(labels).astype(np.int64)
    lab = np.asarray(label_cq).astype(np.int64).copy()
    h0 = int(np.asarray(header_cq))

    uq = np.unique(labels)
    if uq.size < U:
        uniq = np.concatenate([uq, np.full(U - uq.size, uq.min(), np.int64)])
    else:
        uniq = uq[:U]
    cnts = np.array([(labels == v).sum() for v in uniq], np.int64)

    emb_src = np.full(Q, -1, np.int64)   # >=0: row u of uniq means; -1: original
    h = h0 % Q
    for u in range(U):
        y = uniq[u]
        m = lab == y
        i = int(np.argmax(m)) if m.any() else 0
        inval = bool(m.any()) and (i != h)
        emb_src[h] = u
        lab[h] = y
        if inval:
            lab[i] = IGNORE
        h = (h + 1) % Q

    good = lab != IGNORE
    goodidx = np.flatnonzero(good)
    gl = lab[goodidx]
    vals, first = np.unique(gl, return_index=True)
    pos = np.searchsorted(vals, labels)
    assert np.all(vals[np.clip(pos, 0, vals.size - 1)] == labels), \
        "batch label missing from queue"
    xe = goodidx[first[pos]]
    return uniq, cnts, emb_src, good, xe


def _pmajor(v, cols):
    return np.ascontiguousarray(np.asarray(v, np.float32).reshape(cols, 128).T)


def _prepare(inputs, labels, emb_cq, label_cq, header_cq):
    inputs = np.asarray(inputs, np.float32)
    emb_cq = np.asarray(emb_cq, np.float32)
    labels = np.asarray(labels)

    uniq, cnts, emb_src, good, xe = _host_bookkeeping(labels, label_cq, header_cq)

    max_nrm = float(np.sqrt((emb_cq.astype(np.float64) ** 2).sum(axis=1).max()))
    M = OIM_SCALAR * max(1.0, max_nrm) * 1.0000001

    window = emb_src >= 0
    u_slot = np.full(U, -1, np.int64)
    wi = np.flatnonzero(window)
    u_slot[emb_src[wi]] = wi
    u_kept = (u_slot >= 0) & good[np.clip(u_slot, 0, Q - 1)]

    w_idx = emb_src[xe].astype(np.float64)        # -1 for non-window targets
    w_idx[w_idx >= 0] = np.where(
        u_kept[w_idx[w_idx >= 0].astype(np.int64)],
        w_idx[w_idx >= 0], -1.0)
    extra = np.flatnonzero(w_idx < 0)             # handled on host (rare/none)

    # ---- device input layouts ----
    x8 = inputs.astype(F8)
    x8b = np.ascontiguousarray(
        x8.reshape(MT, 128, D).transpose(1, 0, 2).reshape(128, MT * D))
    # xT8[p, kp, i, b] = fp8(x[b, 256*kp + 128*i + p])
    xT8 = np.ascontiguousarray(
        x8.T.reshape(KP, KI, 128, B).transpose(2, 0, 1, 3))

    keep_orig = good & ~window
    embq = (SXE * emb_cq).astype(F8)
    embq[~keep_orig] = 0                          # bad or window -> zero cols
    orig_idx = np.flatnonzero(~window)            # Q-U slots, canonical order
    n_orig0 = QS - W0                             # originals on core 0

    nzero = int((~keep_orig[orig_idx]).sum()) + int((~u_kept).sum())

    rcnt = 1.0 / cnts.astype(np.float64)
    gkr_full = SXE * u_kept.astype(np.float64) * rcnt

    def packed_smalls(core0):
        smalls = np.zeros((128, SC), np.float32)
        smalls[:, _C_RCNT:_C_RCNT + UT] = _pmajor(rcnt, UT)
        smalls[:, _C_GKR:_C_GKR + UT] = _pmajor(
            gkr_full if core0 else np.zeros(U), UT)
        smalls[:, _C_WIDX:_C_WIDX + MT] = _pmajor(w_idx, MT)
        smalls[:, _C_IOTA:_C_IOTA + U] = np.arange(U, dtype=np.float32)[None, :]
        smalls[:, _C_PIDX] = np.arange(128, dtype=np.float32)
        smalls[:, _C_LABF:_C_LABF + MT] = _pmajor(
            labels.astype(np.float64), MT)
        return smalls

    base = {
        "xT8": xT8,
        "x8b": x8b,
        "uniqb": np.ascontiguousarray(
            np.broadcast_to(uniq.astype(BF), (128, U))),
    }
    sm0 = packed_smalls(True)
    smn = packed_smalls(False)

    def to_dmajor(cols):
        # cols: [QS, D] fp8 -> [128, KP, KI, QS] with (p,kp,i,j) layout
        t = np.ascontiguousarray(cols).T          # [D, QS]
        return np.ascontiguousarray(
            t.reshape(KP, KI, 128, QS).transpose(2, 0, 1, 3))

    in_maps = []
    for c in range(N_CORES):
        cols = np.zeros((QS, D), F8)
        if c == 0:
            cols[W0:] = embq[orig_idx[:n_orig0]]
        else:
            sl = orig_idx[n_orig0 + (c - 1) * QS: n_orig0 + c * QS]
            cols[:] = embq[sl]
        in_maps.append({**base, "eT8": to_dmajor(cols),
                        "smalls": sm0 if c == 0 else smn})
    return M, in_maps, extra, xe, nzero


def _combine(res_list, M, extra, xe, nzero, inputs, emb_cq):
    S = np.zeros(B, np.float64)
    for r in res_list:
        S += r["osum"].astype(np.float64).T.reshape(B)
    S -= nzero * np.exp(-np.float64(M))
    t30 = res_list[0]["tco"].astype(np.float64).T.reshape(B)

    if extra.size:  # targets pointing at original (non-window) queue rows
        xb = np.asarray(inputs, np.float64)[extra]
        xb /= np.maximum(np.linalg.norm(xb, axis=1, keepdims=True), 1e-12)
        eb = np.asarray(emb_cq, np.float64)[xe[extra]]
        t30[extra] = OIM_SCALAR * (xb * eb).sum(axis=1)

    loss = np.mean(M + np.log(S) - t30)
    return np.array(loss, dtype=np.float32)


def kernel(inputs, labels, emb_cq, label_cq, age_cq, header_cq):
    from concourse.bass_utils import run_bass_kernel_spmd

    M, in_maps, extra, xe, nzero = _prepare(
        inputs, labels, emb_cq, label_cq, header_cq)

    key = round(M, 9)
    if key not in _PROG_CACHE:
        _PROG_CACHE[key] = _build_program(M)
    nc = _PROG_CACHE[key]

    res = run_bass_kernel_spmd(nc, in_maps, core_ids=list(range(N_CORES)))
    return _combine(res.results, M, extra, xe, nzero, inputs, emb_cq)


# revision 38
# speedup vs baseline: 1.0366x; 1.0366x over previous
"""OIM loss with circular queue — Trainium2 Bass kernel (8 NeuronCores).

Strategy (v7, fp8 DoubleRow end-to-end)
---------------------------------------
loss = mean_b [ M + log S_b - 30*cos(x_b, e_{xe_b}) ],
S_b = sum_{q good} exp(30*cos(x_b, e_q) - M), with e the post-update queue.

Device-side compute per core (tensor-parallel over Q):
  - one-hot label masks on DVE/Pool, per-pid means via fp8-DR matmuls
  - row norms of x: fp8-DR x@x^T per b-tile, diagonal gathered on DVE;
    1/sqrt via tensor_scalar max+pow
  - normalized means -> gated fp8 d-major tiles (PE transpose + cast)
  - big logits matmul in fp8e4 DoubleRow (K=256/pass) into [128,2048] PSUM
    (gated means accumulated onto cols 0:256 in the same group), exp on ACT
    (per-row scale=30*rin/SX, bias=-M), row-sums via a DVE tensor_scalar
    accumulate, target logits gathered from PSUM cols 0:256.

Layout trick: the 256 queue slots rewritten by the circular-queue update
("window") are core 0's first 256 columns (zeros in its eT8; the on-device
means land there through the extra matmul); the other 16128 original slots
fill the rest.  Bad slots (label IGNORE) are zero columns -> each adds
exactly exp(-M), subtracted on the host.  Every batch row's target is a
window slot, so its logit is read from PSUM cols 0:256 on core 0 (cores
1-7 compute garbage there which the host ignores).  Host does the integer
queue bookkeeping, fp8 quantization/transposes of inputs, and the final
log/mean; all O(B*D*Q) FLOPs run on device.
"""

import os
import sys

import numpy as np

for _p in ("/opt/trn_rl_repo", "/root/.axon_site/_ro/trn_rl_repo"):
    if os.path.isdir(_p) and _p not in sys.path:
        sys.path.insert(0, _p)

import ml_dtypes

B, D, Q, U = 4096, 512, 16384, 256
N_CORES = 8
QS = Q // N_CORES          # queue columns per core
W0 = U                     # window block size on core 0's layout
OIM_SCALAR = 30.0
IGNORE = -1
SXE = 16.0                 # fp8 scale for emb/mean operands
MT = B // 128              # 32 b-tiles
KP = 2                     # k-passes of 256 (DoubleRow)
KI = 2                     # interleave factor inside a pass
UT = U // 128              # 2 u-tiles
NQ = QS // 512             # 4 matmul n-chunks per core

# packed small-input column layout
_C_RCNT = 0
_C_GKR = _C_RCNT + UT
_C_WIDX = _C_GKR + UT
_C_IOTA = _C_WIDX + MT
_C_PIDX = _C_IOTA + U
_C_LABF = _C_PIDX + 1
SC = _C_LABF + MT

F8 = ml_dtypes.float8_e4m3
BF = ml_dtypes.bfloat16

_PROG_CACHE = {}


def _build_program(M: float):
    import concourse.bacc as bacc
    import concourse.tile as tile
    from concourse import mybir
    from concourse.masks import make_identity

    f32 = mybir.dt.float32
    bf16 = mybir.dt.bfloat16
    fp8 = mybir.dt.float8e4
    AF = mybir.ActivationFunctionType
    OP = mybir.AluOpType
    DR = mybir.MatmulPerfMode.DoubleRow

    nc = bacc.Bacc("TRN2", target_bir_lowering=False, debug=False,
                   num_devices=N_CORES)

    sm_d = nc.dram_tensor("smalls", [128, SC], f32, kind="ExternalInput").ap()
    unq_d = nc.dram_tensor("uniqb", [128, U], bf16, kind="ExternalInput").ap()
    xT8_d = nc.dram_tensor("xT8", [128, KP, KI, B], fp8, kind="ExternalInput").ap()
    x8b_d = nc.dram_tensor("x8b", [128, MT * D], fp8, kind="ExternalInput").ap()
    eT8_d = nc.dram_tensor("eT8", [128, KP, KI, QS], fp8, kind="ExternalInput").ap()
    osum_d = nc.dram_tensor("osum", [128, MT], f32, kind="ExternalOutput").ap()
    tco_d = nc.dram_tensor("tco", [128, MT], f32, kind="ExternalOutput").ap()

    with tile.TileContext(nc) as tc:
        with (
            tc.tile_pool(name="singles", bufs=1) as singles,
            tc.tile_pool(name="work", bufs=3) as work,
            tc.tile_pool(name="small", bufs=4) as small,
        ):
            ident = singles.tile([128, 128], bf16)
            make_identity(nc, ident)

            # one packed DMA for the small inputs, then big operands
            # interleaved so compute unblocks ASAP
            sm = singles.tile([128, SC], f32)
            nc.sync.dma_start(out=sm, in_=sm_d)
            uniqb = singles.tile([128, U], bf16)
            nc.sync.dma_start(out=uniqb, in_=unq_d)
            rcnt = sm[:, _C_RCNT:_C_RCNT + UT]
            gkr = sm[:, _C_GKR:_C_GKR + UT]
            widx = sm[:, _C_WIDX:_C_WIDX + MT]
            iotab = sm[:, _C_IOTA:_C_IOTA + U]
            pidx = sm[:, _C_PIDX:_C_PIDX + 1]
            labf = sm[:, _C_LABF:_C_LABF + MT]

            BC = B // 4   # b-range per xT8/x8b DMA chunk
            xT8 = singles.tile([128, KP, KI, B], fp8)
            x8b = singles.tile([128, MT, D], fp8)
            mk8 = singles.tile([128, MT, U], fp8)
            eT8 = singles.tile([128, KP, KI, QS], fp8)

            def xT8_chunk(h):
                nc.sync.dma_start(out=xT8[:, :, :, h * BC:(h + 1) * BC],
                                  in_=xT8_d[:, :, :, h * BC:(h + 1) * BC])

            def x8b_chunk(h):
                s = MT // 4 * h
                nc.sync.dma_start(out=x8b[:, s:s + MT // 4, :],
                                  in_=x8b_d[:, s * D:(s + MT // 4) * D])

            for h in range(4):
                x8b_chunk(h)
            for h in range(4):
                xT8_chunk(h)
            for n in range(NQ):
                nc.sync.dma_start(out=eT8[:, :, :, n * 512:(n + 1) * 512],
                                  in_=eT8_d[:, :, :, n * 512:(n + 1) * 512])

            biasM = singles.tile([128, 1], f32)
            nc.vector.memset(biasM, -M)
            # dummy activation at t~0: the single act table (square/copy/exp
            # all co-resident) loads off the critical path
            tinya = singles.tile([128, 1], f32)
            nc.scalar.activation(out=tinya, in_=biasM, func=AF.Square)

            u32 = mybir.dt.uint32

            def emit_rsqrt(dst, src, n, k):
                """dst = k / sqrt(src) on DVE only (no sqrt act table):
                quake-III seed via integer ops + two Newton steps."""
                tu = small.tile([128, n], u32, tag=f"qt{n}")
                # seed = bitcast(0x5f3759df - (bits(src) >> 1)), computed
                # wrap-free as (~ (bits >> 1)) - (0xffffffff - MAGIC)
                nc.vector.tensor_scalar(out=tu, in0=src.bitcast(u32),
                                        scalar1=1, scalar2=None,
                                        op0=OP.logical_shift_right)
                nc.vector.tensor_scalar(out=tu, in0=tu,
                                        scalar1=0xFFFFFFFF, scalar2=None,
                                        op0=OP.bitwise_xor)
                nc.vector.tensor_scalar(out=tu, in0=tu,
                                        scalar1=0xFFFFFFFF - 0x5F3759DF,
                                        scalar2=None, op0=OP.subtract)
                r = tu.bitcast(f32)
                a = small.tile([128, n], f32, tag=f"qa{n}")
                for it in range(2):
                    last = it == 1
                    nc.vector.tensor_tensor(out=a, in0=r, in1=r, op=OP.mult)
                    nc.vector.tensor_tensor(out=a, in0=a, in1=src, op=OP.mult)
                    nc.vector.tensor_scalar(out=a, in0=a,
                                            scalar1=-0.5 * (k if last else 1.0),
                                            scalar2=1.5 * (k if last else 1.0),
                                            op0=OP.mult, op1=OP.add)
                    nc.vector.tensor_tensor(out=dst if last else r,
                                            in0=r, in1=a, op=OP.mult)

            # one-hot masks mk8[:, m, u] = (labels[128m+p] == uniq[u]):
            # early tiles on DVE (idle until the diag loop), rest on Pool
            def emit_mask(m, eng):
                eng.tensor_scalar(out=mk8[:, m, :], in0=uniqb,
                                  scalar1=labf[:, m:m + 1], scalar2=None,
                                  op0=OP.is_equal)
            for m in range(24, MT):
                emit_mask(m, nc.gpsimd)
            for m in range(24):
                emit_mask(m, nc.vector)

            uembT8 = singles.tile([128, KP, KI, U], fp8)   # gated fp8 means^T
            nsq = singles.tile([128, MT], f32)             # |x_b|^2
            rin30 = singles.tile([128, MT], f32)           # 30/(SXE*|x_b|)
            macc = singles.tile([128, UT], f32)            # (sum/cnt)^2 norms
            osum = singles.tile([128, MT], f32)            # sum-exp collector
            tco = singles.tile([128, MT], f32)             # target-cos collector

            # ---------- phase A: masked means + row norms, chunk-wise ------
            with (
                tc.tile_pool(name="psum_u", bufs=1, space="PSUM") as psum_u,
                tc.tile_pool(name="psum_n", bufs=4, space="PSUM") as psum_n,
            ):
                ps_u = [psum_u.tile([128, D], f32, tag=f"uniq{mu}",
                                    name=f"ps_u{mu}") for mu in range(UT)]

                def means_pair(t):
                    for mu in range(UT):
                        nc.tensor.matmul(
                            ps_u[mu],
                            mk8[:, 2 * t:2 * t + 2, mu * 128:(mu + 1) * 128],
                            x8b[:, 2 * t:2 * t + 2, :],
                            start=(t == 0), stop=(t == MT // 2 - 1),
                            perf_mode=DR)

                def diag_tile(m):
                    psn = psum_n.tile([128, 512], f32, tag="psn")
                    dv = psn[:, 0:128]
                    xs = xT8[:, :, :, m * 128:(m + 1) * 128]
                    for kp in range(KP):
                        nc.tensor.matmul(dv, xs[:, kp, :, :], xs[:, kp, :, :],
                                         start=(kp == 0), stop=(kp == KP - 1),
                                         perf_mode=DR)
                    scr = work.tile([128, 128], f32, tag="dscr")
                    nc.vector.scalar_tensor_tensor(
                        out=scr, in0=iotab[:, 0:128], scalar=pidx,
                        in1=dv, op0=OP.is_equal, op1=OP.mult,
                        accum_out=nsq[:, m:m + 1])

                # all means first (x8b chunks lead the DMA pipe)
                for t in range(MT // 2):
                    means_pair(t)

                # mean chain: squared mean norms straight from PSUM (ACT),
                # rmg = SXE*gate*ukeep*rcnt/|mean| (rsqrt on DVE, no sqrt
                # table -> the single exp-capable act table loads once at t~0)
                for mu in range(UT):
                    sq2 = work.tile([128, D], bf16, tag="sq")
                    nc.scalar.activation(out=sq2, in_=ps_u[mu],
                                         func=AF.Square,
                                         scale=rcnt[:, mu:mu + 1],
                                         accum_out=macc[:, mu:mu + 1])

                for m in range(0, 8):
                    diag_tile(m)

                mrcq = small.tile([128, UT], f32, tag="mrcq")
                emit_rsqrt(mrcq, macc, UT, 1.0)
                rmg = small.tile([128, UT], f32, tag="rmg")
                nc.vector.tensor_tensor(out=rmg, in0=mrcq, in1=gkr,
                                        op=OP.mult)

                for m in range(8, 24):
                    diag_tile(m)

                # normalized gated means (ACT copy) -> transpose -> fp8 (ACT)
                mng_t = [singles.tile([128, D], bf16, name=f"mng{mu}")
                         for mu in range(UT)]
                for mu in range(UT):
                    nc.scalar.activation(out=mng_t[mu], in_=ps_u[mu],
                                         func=AF.Copy,
                                         scale=rmg[:, mu:mu + 1])
                for mu in range(UT):
                    pst = psum_n.tile([128, D], bf16, tag="pst", bufs=2)
                    for kd in range(D // 128):
                        nc.tensor.transpose(pst[:, kd * 128:(kd + 1) * 128],
                                            mng_t[mu][:, kd * 128:(kd + 1) * 128],
                                            ident)
                    nc.scalar.activation(
                        out=uembT8[:, :, :, mu * 128:(mu + 1) * 128],
                        in_=pst, func=AF.Copy)

                for m in range(24, MT):
                    diag_tile(m)

                # rin30 = (30/SXE)/sqrt(nsq), all-DVE
                emit_rsqrt(rin30, nsq, MT, OIM_SCALAR / SXE)

            # ---------- phase C: logits + exp + sums + target gather -------
            with tc.tile_pool(name="psum_l", bufs=2, space="PSUM") as psum_l:
                for m in range(MT):
                    pl = psum_l.tile([128, NQ * 512], f32, tag="pl")
                    xs = xT8[:, :, :, m * 128:(m + 1) * 128]
                    for n in (1, 2, 3, 0):
                        for kp in range(KP):
                            nc.tensor.matmul(
                                pl[:, n * 512:(n + 1) * 512],
                                xs[:, kp, :, :],
                                eT8[:, kp, :, n * 512:(n + 1) * 512],
                                start=(kp == 0),
                                stop=(kp == KP - 1 and n != 0),
                                perf_mode=DR)
                    # gated means accumulate onto cols 0:U, same group as the
                    # n=0 chunk (eT8 cols 0:U are zero on core 0; uembT8 is
                    # zero on cores 1-7)
                    for kp in range(KP):
                        nc.tensor.matmul(
                            pl[:, 0:U], xs[:, kp, :, :], uembT8[:, kp, :, :],
                            start=False, stop=(kp == KP - 1),
                            perf_mode=DR, skip_group_check=True)
                    # target logit: window cols live at 0:U (core 0 layout)
                    scr = work.tile([128, U], f32, tag="scr")
                    nc.vector.scalar_tensor_tensor(
                        out=scr, in0=iotab, scalar=widx[:, m:m + 1],
                        in1=pl[:, 0:U], op0=OP.is_equal, op1=OP.mult,
                        accum_out=tco[:, m:m + 1])
                    ex = work.tile([128, NQ * 512], bf16, tag="ex", bufs=2)
                    nc.scalar.activation(out=ex, in_=pl, func=AF.Exp,
                                         bias=biasM, scale=rin30[:, m:m + 1])
                    nc.vector.tensor_scalar(out=ex, in0=ex, scalar1=1.0,
                                            scalar2=0.0, op0=OP.mult,
                                            op1=OP.add,
                                            accum_out=osum[:, m:m + 1])
                    if m == MT // 2 - 1:
                        nc.sync.dma_start(out=osum_d[:, 0:MT // 2],
                                          in_=osum[:, 0:MT // 2])
                nc.vector.tensor_tensor(out=tco, in0=tco, in1=rin30,
                                        op=OP.mult)

            nc.sync.dma_start(out=osum_d[:, MT // 2:], in_=osum[:, MT // 2:])
            nc.sync.dma_start(out=tco_d, in_=tco)

    nc.compile()
    return nc


def _host_bookkeeping(labels, label_cq, header_cq):
    """Mirror the reference's integer-only queue-update semantics."""
    labels = np.asarray(labels).astype(np.int64)
    lab = np.asarray(label_cq).astype(np.int64).copy()
    h0 = int(np.asarray(header_cq))

    uq = np.unique(labels)
    if uq.size < U:
        uniq = np.concatenate([uq, np.full(U - uq.size, uq.min(), np.int64)])
    else:
        uniq = uq[:U]
    cnts = np.array([(labels == v).sum() for v in uniq], np.int64)

    emb_src = np.full(Q, -1, np.int64)   # >=0: row u of uniq means; -1: original
    h = h0 % Q
    for u in range(U):
        y = uniq[u]
        m = lab == y
        i = int(np.argmax(m)) if m.any() else 0
        inval = bool(m.any()) and (i != h)
        emb_src[h] = u
        lab[h] = y
        if inval:
            lab[i] = IGNORE
        h = (h + 1) % Q

    good = lab != IGNORE
    goodidx = np.flatnonzero(good)
    gl = lab[goodidx]
    vals, first = np.unique(gl, return_index=True)
    pos = np.searchsorted(vals, labels)
    assert np.all(vals[np.clip(pos, 0, vals.size - 1)] == labels), \
        "batch label missing from queue"
    xe = goodidx[first[pos]]
    return uniq, cnts, emb_src, good, xe


def _pmajor(v, cols):
    return np.ascontiguousarray(np.asarray(v, np.float32).reshape(cols, 128).T)


def _prepare(inputs, labels, emb_cq, label_cq, header_cq):
    inputs = np.asarray(inputs, np.float32)
    emb_cq = np.asarray(emb_cq, np.float32)
    labels = np.asarray(labels)

    uniq, cnts, emb_src, good, xe = _host_bookkeeping(labels, label_cq, header_cq)

    max_nrm = float(np.sqrt((emb_cq.astype(np.float64) ** 2).sum(axis=1).max()))
    M = OIM_SCALAR * max(1.0, max_nrm) * 1.0000001

    window = emb_src >= 0
    u_slot = np.full(U, -1, np.int64)
    wi = np.flatnonzero(window)
    u_slot[emb_src[wi]] = wi
    u_kept = (u_slot >= 0) & good[np.clip(u_slot, 0, Q - 1)]

    w_idx = emb_src[xe].astype(np.float64)        # -1 for non-window targets
    w_idx[w_idx >= 0] = np.where(
        u_kept[w_idx[w_idx >= 0].astype(np.int64)],
        w_idx[w_idx >= 0], -1.0)
    extra = np.flatnonzero(w_idx < 0)             # handled on host (rare/none)

    # ---- device input layouts ----
    x8 = inputs.astype(F8)
    x8b = np.ascontiguousarray(
        x8.reshape(MT, 128, D).transpose(1, 0, 2).reshape(128, MT * D))
    # xT8[p, kp, i, b] = fp8(x[b, 256*kp + 128*i + p])
    xT8 = np.ascontiguousarray(
        x8.T.reshape(KP, KI, 128, B).transpose(2, 0, 1, 3))

    keep_orig = good & ~window
    embq = (SXE * emb_cq).astype(F8)
    embq[~keep_orig] = 0                          # bad or window -> zero cols
    orig_idx = np.flatnonzero(~window)            # Q-U slots, canonical order
    n_orig0 = QS - W0                             # originals on core 0

    nzero = int((~keep_orig[orig_idx]).sum()) + int((~u_kept).sum())

    rcnt = 1.0 / cnts.astype(np.float64)
    gkr_full = SXE * u_kept.astype(np.float64) * rcnt

    def packed_smalls(core0):
        smalls = np.zeros((128, SC), np.float32)
        smalls[:, _C_RCNT:_C_RCNT + UT] = _pmajor(rcnt, UT)
        smalls[:, _C_GKR:_C_GKR + UT] = _pmajor(
            gkr_full if core0 else np.zeros(U), UT)
        smalls[:, _C_WIDX:_C_WIDX + MT] = _pmajor(w_idx, MT)
        smalls[:, _C_IOTA:_C_IOTA + U] = np.arange(U, dtype=np.float32)[None, :]
        smalls[:, _C_PIDX] = np.arange(128, dtype=np.float32)
        smalls[:, _C_LABF:_C_LABF + MT] = _pmajor(
            labels.astype(np.float64), MT)
        return smalls

    base = {
        "xT8": xT8,
        "x8b": x8b,
        "uniqb": np.ascontiguousarray(
            np.broadcast_to(uniq.astype(BF), (128, U))),
    }
    sm0 = packed_smalls(True)
    smn = packed_smalls(False)

    def to_dmajor(cols):
        # cols: [QS, D] fp8 -> [128, KP, KI, QS] with (p,kp,i,j) layout
        t = np.ascontiguousarray(cols).T          # [D, QS]
        return np.ascontiguousarray(
            t.reshape(KP, KI, 128, QS).transpose(2, 0, 1, 3))

    in_maps = []
    for c in range(N_CORES):
        cols = np.zeros((QS, D), F8)
        if c == 0:
            cols[W0:] = embq[orig_idx[:n_orig0]]
        else:
            sl = orig_idx[n_orig0 + (c - 1) * QS: n_orig0 + c * QS]
            cols[:] = embq[sl]
        in_maps.append({**base, "eT8": to_dmajor(cols),
                        "smalls": sm0 if c == 0 else smn})
    return M, in_maps, extra, xe, nzero


def _combine(res_list, M, extra, xe, nzero, inputs, emb_cq):
    S = np.zeros(B, np.float64)
    for r in res_list:
        S += r["osum"].astype(np.float64).T.reshape(B)
    S -= nzero * np.exp(-np.float64(M))
    t30 = res_list[0]["tco"].astype(np.float64).T.reshape(B)

    if extra.size:  # targets pointing at original (non-window) queue rows
        xb = np.asarray(inputs, np.float64)[extra]
        xb /= np.maximum(np.linalg.norm(xb, axis=1, keepdims=True), 1e-12)
        eb = np.asarray(emb_cq, np.float64)[xe[extra]]
        t30[extra] = OIM_SCALAR * (xb * eb).sum(axis=1)

    loss = np.mean(M + np.log(S) - t30)
    return np.array(loss, dtype=np.float32)


def kernel(inputs, labels, emb_cq, label_cq, age_cq, header_cq):
    from concourse.bass_utils import run_bass_kernel_spmd

    M, in_maps, extra, xe, nzero = _prepare(
        inputs, labels, emb_cq, label_cq, header_cq)

    key = round(M, 9)
    if key not in _PROG_CACHE:
        _PROG_CACHE[key] = _build_program(M)
    nc = _PROG_CACHE[key]

    res = run_bass_kernel_spmd(nc, in_maps, core_ids=list(range(N_CORES)))
    return _combine(res.results, M, extra, xe, nzero, inputs, emb_cq)


# revision 55
# speedup vs baseline: 1.0427x; 1.0058x over previous
"""OIM loss with circular queue — Trainium2 Bass kernel (8 NeuronCores).

Strategy (v7, fp8 DoubleRow end-to-end)
---------------------------------------
loss = mean_b [ M + log S_b - 30*cos(x_b, e_{xe_b}) ],
S_b = sum_{q good} exp(30*cos(x_b, e_q) - M), with e the post-update queue.

Device-side compute per core (tensor-parallel over Q):
  - one-hot label masks on DVE/Pool, per-pid means via fp8-DR matmuls
  - row norms of x: fp8-DR x@x^T per b-tile, diagonal gathered on DVE;
    1/sqrt via tensor_scalar max+pow
  - normalized means -> gated fp8 d-major tiles (PE transpose + cast)
  - big logits matmul in fp8e4 DoubleRow (K=256/pass) into [128,2048] PSUM
    (gated means accumulated onto cols 0:256 in the same group), exp on ACT
    (per-row scale=30*rin/SX, bias=-M), row-sums via a DVE tensor_scalar
    accumulate, target logits gathered from PSUM cols 0:256.

Layout trick: the 256 queue slots rewritten by the circular-queue update
("window") are core 0's first 256 columns (zeros in its eT8; the on-device
means land there through the extra matmul); the other 16128 original slots
fill the rest.  Bad slots (label IGNORE) are zero columns -> each adds
exactly exp(-M), subtracted on the host.  Every batch row's target is a
window slot, so its logit is read from PSUM cols 0:256 on core 0 (cores
1-7 compute garbage there which the host ignores).  Host does the integer
queue bookkeeping, fp8 quantization/transposes of inputs, and the final
log/mean; all O(B*D*Q) FLOPs run on device.
"""

import os
import sys

import numpy as np

for _p in ("/opt/trn_rl_repo", "/root/.axon_site/_ro/trn_rl_repo"):
    if os.path.isdir(_p) and _p not in sys.path:
        sys.path.insert(0, _p)

import ml_dtypes

B, D, Q, U = 4096, 512, 16384, 256
N_CORES = 8
QS = Q // N_CORES          # queue columns per core
W0 = U                     # window block size on core 0's layout
OIM_SCALAR = 30.0
IGNORE = -1
SXE = 16.0                 # fp8 scale for emb/mean operands
MT = B // 128              # 32 b-tiles
KP = 2                     # k-passes of 256 (DoubleRow)
KI = 2                     # interleave factor inside a pass
UT = U // 128              # 2 u-tiles
NQ = QS // 512             # 4 matmul n-chunks per core

# packed small-input column layout
_C_RCNT = 0
_C_GKR = _C_RCNT + UT
_C_WIDX = _C_GKR + UT
_C_IOTA = _C_WIDX + MT
_C_PIDX = _C_IOTA + U
_C_LABF = _C_PIDX + 1
SC = _C_LABF + MT

F8 = ml_dtypes.float8_e4m3
BF = ml_dtypes.bfloat16

_PROG_CACHE = {}


def _build_program(M: float):
    import concourse.bacc as bacc
    import concourse.tile as tile
    from concourse import mybir
    from concourse.masks import make_identity

    f32 = mybir.dt.float32
    bf16 = mybir.dt.bfloat16
    fp8 = mybir.dt.float8e4
    AF = mybir.ActivationFunctionType
    OP = mybir.AluOpType
    DR = mybir.MatmulPerfMode.DoubleRow

    nc = bacc.Bacc("TRN2", target_bir_lowering=False, debug=False,
                   num_devices=N_CORES)

    sm_d = nc.dram_tensor("smalls", [128, SC], f32, kind="ExternalInput").ap()
    unq_d = nc.dram_tensor("uniqb", [128, U], bf16, kind="ExternalInput").ap()
    xT8_d = nc.dram_tensor("xT8", [128, KP, KI, B], fp8, kind="ExternalInput").ap()
    x8b_d = nc.dram_tensor("x8b", [128, MT * D], fp8, kind="ExternalInput").ap()
    eT8_d = nc.dram_tensor("eT8", [128, KP, KI, QS], fp8, kind="ExternalInput").ap()
    osum_d = nc.dram_tensor("osum", [128, MT], f32, kind="ExternalOutput").ap()
    tco_d = nc.dram_tensor("tco", [128, MT], f32, kind="ExternalOutput").ap()

    with tile.TileContext(nc) as tc:
        with (
            tc.tile_pool(name="singles", bufs=1) as singles,
            tc.tile_pool(name="work", bufs=3) as work,
            tc.tile_pool(name="small", bufs=4) as small,
        ):
            ident = singles.tile([128, 128], bf16)
            make_identity(nc, ident)

            # one packed DMA for the small inputs, then big operands
            # interleaved so compute unblocks ASAP
            sm = singles.tile([128, SC], f32)
            nc.sync.dma_start(out=sm, in_=sm_d)
            uniqb = singles.tile([128, U], bf16)
            nc.sync.dma_start(out=uniqb, in_=unq_d)
            rcnt = sm[:, _C_RCNT:_C_RCNT + UT]
            gkr = sm[:, _C_GKR:_C_GKR + UT]
            widx = sm[:, _C_WIDX:_C_WIDX + MT]
            iotab = sm[:, _C_IOTA:_C_IOTA + U]
            pidx = sm[:, _C_PIDX:_C_PIDX + 1]
            labf = sm[:, _C_LABF:_C_LABF + MT]

            BC = B // 4   # b-range per xT8/x8b DMA chunk
            xT8 = singles.tile([128, KP, KI, B], fp8)
            x8b = singles.tile([128, MT, D], fp8)
            mk8 = singles.tile([128, MT, U], fp8)
            eT8 = singles.tile([128, KP, KI, QS], fp8)

            def xT8_chunk(h):
                nc.sync.dma_start(out=xT8[:, :, :, h * BC:(h + 1) * BC],
                                  in_=xT8_d[:, :, :, h * BC:(h + 1) * BC])

            def x8b_chunk(h):
                s = MT // 4 * h
                nc.sync.dma_start(out=x8b[:, s:s + MT // 4, :],
                                  in_=x8b_d[:, s * D:(s + MT // 4) * D])

            for h in range(4):
                x8b_chunk(h)
            for h in range(4):
                xT8_chunk(h)
            for n in range(NQ):
                nc.sync.dma_start(out=eT8[:, :, :, n * 512:(n + 1) * 512],
                                  in_=eT8_d[:, :, :, n * 512:(n + 1) * 512])

            biasM = singles.tile([128, 1], f32)
            nc.vector.memset(biasM, -M)
            # dummy activation at t~0: the single act table (square/copy/exp
            # all co-resident) loads off the critical path
            tinya = singles.tile([128, 1], f32)
            nc.scalar.activation(out=tinya, in_=biasM, func=AF.Square)

            u32 = mybir.dt.uint32

            def emit_rsqrt(dst, src, n, k, eng):
                """dst = k / sqrt(src), elementwise on one engine (no sqrt
                act table): quake-III seed via integer ops + 2 Newton steps."""
                tu = small.tile([128, n], u32, tag=f"qt{n}")
                # seed = bitcast(0x5f3759df - (bits(src) >> 1)), computed
                # wrap-free as (~ (bits >> 1)) - (0xffffffff - MAGIC)
                eng.tensor_scalar(out=tu, in0=src.bitcast(u32),
                                  scalar1=1, scalar2=None,
                                  op0=OP.logical_shift_right)
                eng.tensor_scalar(out=tu, in0=tu,
                                  scalar1=0xFFFFFFFF, scalar2=None,
                                  op0=OP.bitwise_xor)
                eng.tensor_scalar(out=tu, in0=tu,
                                  scalar1=0xFFFFFFFF - 0x5F3759DF,
                                  scalar2=None, op0=OP.subtract)
                r = tu.bitcast(f32)
                a = small.tile([128, n], f32, tag=f"qa{n}")
                for it in range(2):
                    last = it == 1
                    eng.tensor_tensor(out=a, in0=r, in1=r, op=OP.mult)
                    eng.tensor_tensor(out=a, in0=a, in1=src, op=OP.mult)
                    eng.tensor_scalar(out=a, in0=a,
                                      scalar1=-0.5 * (k if last else 1.0),
                                      scalar2=1.5 * (k if last else 1.0),
                                      op0=OP.mult, op1=OP.add)
                    eng.tensor_tensor(out=dst if last else r,
                                      in0=r, in1=a, op=OP.mult)

            # one-hot masks mk8[:, m, u] = (labels[128m+p] == uniq[u]):
            # early tiles on DVE (idle until the diag loop), rest on Pool
            def emit_mask(m, eng):
                eng.tensor_scalar(out=mk8[:, m, :], in0=uniqb,
                                  scalar1=labf[:, m:m + 1], scalar2=None,
                                  op0=OP.is_equal)
            for m in range(24, MT):
                emit_mask(m, nc.gpsimd)
            for m in range(24):
                emit_mask(m, nc.vector)

            uembT8 = singles.tile([128, KP, KI, U], fp8)   # gated fp8 means^T
            nsq = singles.tile([128, MT], f32)             # |x_b|^2
            rin30 = singles.tile([128, MT], f32)           # 30/(SXE*|x_b|)
            macc = singles.tile([128, UT], f32)            # (sum/cnt)^2 norms
            osum = singles.tile([128, MT], f32)            # sum-exp collector
            tco = singles.tile([128, MT], f32)             # target-cos collector

            # ---------- phase A: masked means + row norms, chunk-wise ------
            with (
                tc.tile_pool(name="psum_u", bufs=1, space="PSUM") as psum_u,
                tc.tile_pool(name="psum_n", bufs=5, space="PSUM") as psum_n,
            ):
                ps_u = [psum_u.tile([128, D], f32, tag=f"uniq{mu}",
                                    name=f"ps_u{mu}") for mu in range(UT)]

                def means_pair(t):
                    for mu in range(UT):
                        nc.tensor.matmul(
                            ps_u[mu],
                            mk8[:, 2 * t:2 * t + 2, mu * 128:(mu + 1) * 128],
                            x8b[:, 2 * t:2 * t + 2, :],
                            start=(t == 0), stop=(t == MT // 2 - 1),
                            perf_mode=DR)

                def diag_tile(m):
                    psn = psum_n.tile([128, 512], f32, tag="psn")
                    dv = psn[:, 0:128]
                    xs = xT8[:, :, :, m * 128:(m + 1) * 128]
                    for kp in range(KP):
                        nc.tensor.matmul(dv, xs[:, kp, :, :], xs[:, kp, :, :],
                                         start=(kp == 0), stop=(kp == KP - 1),
                                         perf_mode=DR)
                    scr = work.tile([128, 128], f32, tag="dscr")
                    nc.vector.scalar_tensor_tensor(
                        out=scr, in0=iotab[:, 0:128], scalar=pidx,
                        in1=dv, op0=OP.is_equal, op1=OP.mult,
                        accum_out=nsq[:, m:m + 1])

                # all means first (x8b chunks lead the DMA pipe)
                for t in range(MT // 2):
                    means_pair(t)

                # mean chain: squared mean norms straight from PSUM (ACT),
                # rmg = SXE*gate*ukeep*rcnt/|mean| (rsqrt on DVE, no sqrt
                # table -> the single exp-capable act table loads once at t~0)
                for mu in range(UT):
                    sq2 = work.tile([128, D], bf16, tag="sq")
                    nc.scalar.activation(out=sq2, in_=ps_u[mu],
                                         func=AF.Square,
                                         scale=rcnt[:, mu:mu + 1],
                                         accum_out=macc[:, mu:mu + 1])

                for m in range(0, 8):
                    diag_tile(m)

                mrcq = small.tile([128, UT], f32, tag="mrcq")
                emit_rsqrt(mrcq, macc, UT, 1.0, nc.vector)
                rmg = small.tile([128, UT], f32, tag="rmg")
                nc.vector.tensor_tensor(out=rmg, in0=mrcq, in1=gkr,
                                        op=OP.mult)

                for m in range(8, 24):
                    diag_tile(m)

                # normalized gated means (ACT copy) -> transpose -> fp8 (ACT)
                mng_t = [singles.tile([128, D], bf16, name=f"mng{mu}")
                         for mu in range(UT)]
                for mu in range(UT):
                    nc.scalar.activation(out=mng_t[mu], in_=ps_u[mu],
                                         func=AF.Copy,
                                         scale=rmg[:, mu:mu + 1])
                for mu in range(UT):
                    pst = psum_n.tile([128, D], bf16, tag="pst", bufs=1)
                    for kd in range(D // 128):
                        nc.tensor.transpose(pst[:, kd * 128:(kd + 1) * 128],
                                            mng_t[mu][:, kd * 128:(kd + 1) * 128],
                                            ident)
                    nc.scalar.activation(
                        out=uembT8[:, :, :, mu * 128:(mu + 1) * 128],
                        in_=pst, func=AF.Copy)

                emit_rsqrt(rin30[:, 0:24], nsq[:, 0:24], 24,
                           OIM_SCALAR / SXE, nc.vector)

                for m in range(24, MT):
                    diag_tile(m)

                emit_rsqrt(rin30[:, 24:MT], nsq[:, 24:MT], 8,
                           OIM_SCALAR / SXE, nc.vector)

            # ---------- phase C: logits + exp + sums + target gather -------
            with tc.tile_pool(name="psum_l", bufs=2, space="PSUM") as psum_l:
                for m in range(MT):
                    pl = psum_l.tile([128, NQ * 512], f32, tag="pl")
                    xs = xT8[:, :, :, m * 128:(m + 1) * 128]
                    for n in (1, 2, 3, 0):
                        for kp in range(KP):
                            nc.tensor.matmul(
                                pl[:, n * 512:(n + 1) * 512],
                                xs[:, kp, :, :],
                                eT8[:, kp, :, n * 512:(n + 1) * 512],
                                start=(kp == 0),
                                stop=(kp == KP - 1 and n != 0),
                                perf_mode=DR)
                    # gated means accumulate onto cols 0:U, same group as the
                    # n=0 chunk (eT8 cols 0:U are zero on core 0; uembT8 is
                    # zero on cores 1-7)
                    for kp in range(KP):
                        nc.tensor.matmul(
                            pl[:, 0:U], xs[:, kp, :, :], uembT8[:, kp, :, :],
                            start=False, stop=(kp == KP - 1),
                            perf_mode=DR, skip_group_check=True)
                    # target logit: window cols live at 0:U (core 0 layout)
                    scr = work.tile([128, U], f32, tag="scr")
                    nc.vector.scalar_tensor_tensor(
                        out=scr, in0=iotab, scalar=widx[:, m:m + 1],
                        in1=pl[:, 0:U], op0=OP.is_equal, op1=OP.mult,
                        accum_out=tco[:, m:m + 1])
                    ex = work.tile([128, NQ * 512], bf16, tag="ex", bufs=2)
                    nc.scalar.activation(out=ex, in_=pl, func=AF.Exp,
                                         bias=biasM, scale=rin30[:, m:m + 1])
                    nc.vector.tensor_scalar(out=ex, in0=ex, scalar1=1.0,
                                            scalar2=0.0, op0=OP.mult,
                                            op1=OP.add,
                                            accum_out=osum[:, m:m + 1])
                    if m == MT // 2 - 1:
                        nc.sync.dma_start(out=osum_d[:, 0:MT // 2],
                                          in_=osum[:, 0:MT // 2])
                nc.vector.tensor_tensor(out=tco, in0=tco, in1=rin30,
                                        op=OP.mult)

            nc.sync.dma_start(out=osum_d[:, MT // 2:], in_=osum[:, MT // 2:])
            nc.sync.dma_start(out=tco_d, in_=tco)

    nc.compile()
    return nc


def _host_bookkeeping(labels, label_cq, header_cq):
    """Mirror the reference's integer-only queue-update semantics."""
    labels = np.asarray(labels).astype(np.int64)
    lab = np.asarray(label_cq).astype(np.int64).copy()
    h0 = int(np.asarray(header_cq))

    uq = np.unique(labels)
    if uq.size < U:
        uniq = np.concatenate([uq, np.full(U - uq.size, uq.min(), np.int64)])
    else:
        uniq = uq[:U]
    cnts = np.array([(labels == v).sum() for v in uniq], np.int64)

    emb_src = np.full(Q, -1, np.int64)   # >=0: row u of uniq means; -1: original
    h = h0 % Q
    for u in range(U):
        y = uniq[u]
        m = lab == y
        i = int(np.argmax(m)) if m.any() else 0
        inval = bool(m.any()) and (i != h)
        emb_src[h] = u
        lab[h] = y
        if inval:
            lab[i] = IGNORE
        h = (h + 1) % Q

    good = lab != IGNORE
    goodidx = np.flatnonzero(good)
    gl = lab[goodidx]
    vals, first = np.unique(gl, return_index=True)
    pos = np.searchsorted(vals, labels)
    assert np.all(vals[np.clip(pos, 0, vals.size - 1)] == labels), \
        "batch label missing from queue"
    xe = goodidx[first[pos]]
    return uniq, cnts, emb_src, good, xe


def _pmajor(v, cols):
    return np.ascontiguousarray(np.asarray(v, np.float32).reshape(cols, 128).T)


def _prepare(inputs, labels, emb_cq, label_cq, header_cq):
    inputs = np.asarray(inputs, np.float32)
    emb_cq = np.asarray(emb_cq, np.float32)
    labels = np.asarray(labels)

    uniq, cnts, emb_src, good, xe = _host_bookkeeping(labels, label_cq, header_cq)

    max_nrm = float(np.sqrt((emb_cq.astype(np.float64) ** 2).sum(axis=1).max()))
    M = OIM_SCALAR * max(1.0, max_nrm) * 1.0000001

    window = emb_src >= 0
    u_slot = np.full(U, -1, np.int64)
    wi = np.flatnonzero(window)
    u_slot[emb_src[wi]] = wi
    u_kept = (u_slot >= 0) & good[np.clip(u_slot, 0, Q - 1)]

    w_idx = emb_src[xe].astype(np.float64)        # -1 for non-window targets
    w_idx[w_idx >= 0] = np.where(
        u_kept[w_idx[w_idx >= 0].astype(np.int64)],
        w_idx[w_idx >= 0], -1.0)
    extra = np.flatnonzero(w_idx < 0)             # handled on host (rare/none)

    # ---- device input layouts ----
    x8 = inputs.astype(F8)
    x8b = np.ascontiguousarray(
        x8.reshape(MT, 128, D).transpose(1, 0, 2).reshape(128, MT * D))
    # xT8[p, kp, i, b] = fp8(x[b, 256*kp + 128*i + p])
    xT8 = np.ascontiguousarray(
        x8.T.reshape(KP, KI, 128, B).transpose(2, 0, 1, 3))

    keep_orig = good & ~window
    embq = (SXE * emb_cq).astype(F8)
    embq[~keep_orig] = 0                          # bad or window -> zero cols
    orig_idx = np.flatnonzero(~window)            # Q-U slots, canonical order
    n_orig0 = QS - W0                             # originals on core 0

    nzero = int((~keep_orig[orig_idx]).sum()) + int((~u_kept).sum())

    rcnt = 1.0 / cnts.astype(np.float64)
    gkr_full = SXE * u_kept.astype(np.float64) * rcnt

    def packed_smalls(core0):
        smalls = np.zeros((128, SC), np.float32)
        smalls[:, _C_RCNT:_C_RCNT + UT] = _pmajor(rcnt, UT)
        smalls[:, _C_GKR:_C_GKR + UT] = _pmajor(
            gkr_full if core0 else np.zeros(U), UT)
        smalls[:, _C_WIDX:_C_WIDX + MT] = _pmajor(w_idx, MT)
        smalls[:, _C_IOTA:_C_IOTA + U] = np.arange(U, dtype=np.float32)[None, :]
        smalls[:, _C_PIDX] = np.arange(128, dtype=np.float32)
        smalls[:, _C_LABF:_C_LABF + MT] = _pmajor(
            labels.astype(np.float64), MT)
        return smalls

    base = {
        "xT8": xT8,
        "x8b": x8b,
        "uniqb": np.ascontiguousarray(
            np.broadcast_to(uniq.astype(BF), (128, U))),
    }
    sm0 = packed_smalls(True)
    smn = packed_smalls(False)

    def to_dmajor(cols):
        # cols: [QS, D] fp8 -> [128, KP, KI, QS] with (p,kp,i,j) layout
        t = np.ascontiguousarray(cols).T          # [D, QS]
        return np.ascontiguousarray(
            t.reshape(KP, KI, 128, QS).transpose(2, 0, 1, 3))

    in_maps = []
    for c in range(N_CORES):
        cols = np.zeros((QS, D), F8)
        if c == 0:
            cols[W0:] = embq[orig_idx[:n_orig0]]
        else:
            sl = orig_idx[n_orig0 + (c - 1) * QS: n_orig0 + c * QS]
            cols[:] = embq[sl]
        in_maps.append({**base, "eT8": to_dmajor(cols),
                        "smalls": sm0 if c == 0 else smn})
    return M, in_maps, extra, xe, nzero


def _combine(res_list, M, extra, xe, nzero, inputs, emb_cq):
    S = np.zeros(B, np.float64)
    for r in res_list:
        S += r["osum"].astype(np.float64).T.reshape(B)
    S -= nzero * np.exp(-np.float64(M))
    t30 = res_list[0]["tco"].astype(np.float64).T.reshape(B)

    if extra.size:  # targets pointing at original (non-window) queue rows
        xb = np.asarray(inputs, np.float64)[extra]
        xb /= np.maximum(np.linalg.norm(xb, axis=1, keepdims=True), 1e-12)
        eb = np.asarray(emb_cq, np.float64)[xe[extra]]
        t30[extra] = OIM_SCALAR * (xb * eb).sum(axis=1)

    loss = np.mean(M + np.log(S) - t30)
    return np.array(loss, dtype=np.float32)


def kernel(inputs, labels, emb_cq, label_cq, age_cq, header_cq):
    from concourse.bass_utils import run_bass_kernel_spmd

    M, in_maps, extra, xe, nzero = _prepare(
        inputs, labels, emb_cq, label_cq, header_cq)

    key = round(M, 9)
    if key not in _PROG_CACHE:
        _PROG_CACHE[key] = _build_program(M)
    nc = _PROG_CACHE[key]

    res = run_bass_kernel_spmd(nc, in_maps, core_ids=list(range(N_CORES)))
    return _combine(res.results, M, extra, xe, nzero, inputs, emb_cq)


# revision 60
# speedup vs baseline: 1.0531x; 1.0100x over previous
"""OIM loss with circular queue — Trainium2 Bass kernel (8 NeuronCores).

Strategy (v7, fp8 DoubleRow end-to-end)
---------------------------------------
loss = mean_b [ M + log S_b - 30*cos(x_b, e_{xe_b}) ],
S_b = sum_{q good} exp(30*cos(x_b, e_q) - M), with e the post-update queue.

Device-side compute per core (tensor-parallel over Q):
  - one-hot label masks on DVE/Pool, per-pid means via fp8-DR matmuls
  - row norms of x: fp8-DR x@x^T per b-tile, diagonal gathered on DVE;
    1/sqrt via tensor_scalar max+pow
  - normalized means -> gated fp8 d-major tiles (PE transpose + cast)
  - big logits matmul in fp8e4 DoubleRow (K=256/pass) into [128,2048] PSUM
    (gated means accumulated onto cols 0:256 in the same group), exp on ACT
    (per-row scale=30*rin/SX, bias=-M), row-sums via a DVE tensor_scalar
    accumulate, target logits gathered from PSUM cols 0:256.

Layout trick: the 256 queue slots rewritten by the circular-queue update
("window") are core 0's first 256 columns (zeros in its eT8; the on-device
means land there through the extra matmul); the other 16128 original slots
fill the rest.  Bad slots (label IGNORE) are zero columns -> each adds
exactly exp(-M), subtracted on the host.  Every batch row's target is a
window slot, so its logit is read from PSUM cols 0:256 on core 0 (cores
1-7 compute garbage there which the host ignores).  Host does the integer
queue bookkeeping, fp8 quantization/transposes of inputs, and the final
log/mean; all O(B*D*Q) FLOPs run on device.
"""

import os
import sys

import numpy as np

for _p in ("/opt/trn_rl_repo", "/root/.axon_site/_ro/trn_rl_repo"):
    if os.path.isdir(_p) and _p not in sys.path:
        sys.path.insert(0, _p)

import ml_dtypes

B, D, Q, U = 4096, 512, 16384, 256
N_CORES = 8
QS = Q // N_CORES          # queue columns per core
W0 = U                     # window block size on core 0's layout
OIM_SCALAR = 30.0
IGNORE = -1
SXE = 16.0                 # fp8 scale for emb/mean operands
MT = B // 128              # 32 b-tiles
KP = 2                     # k-passes of 256 (DoubleRow)
KI = 2                     # interleave factor inside a pass
UT = U // 128              # 2 u-tiles
NQ = QS // 512             # 4 matmul n-chunks per core

# packed small-input column layout
_C_RCNT = 0
_C_GKR = _C_RCNT + UT
_C_WIDX = _C_GKR + UT
_C_IOTA = _C_WIDX + MT
_C_PIDX = _C_IOTA + U
_C_LABF = _C_PIDX + 1
SC = _C_LABF + MT

F8 = ml_dtypes.float8_e4m3
BF = ml_dtypes.bfloat16

_PROG_CACHE = {}


def _build_program(M: float):
    import concourse.bacc as bacc
    import concourse.tile as tile
    from concourse import mybir
    from concourse.masks import make_identity

    f32 = mybir.dt.float32
    bf16 = mybir.dt.bfloat16
    fp8 = mybir.dt.float8e4
    AF = mybir.ActivationFunctionType
    OP = mybir.AluOpType
    DR = mybir.MatmulPerfMode.DoubleRow

    nc = bacc.Bacc("TRN2", target_bir_lowering=False, debug=False,
                   num_devices=N_CORES)

    sm_d = nc.dram_tensor("smalls", [128, SC], f32, kind="ExternalInput").ap()
    unq_d = nc.dram_tensor("uniqb", [128, U], bf16, kind="ExternalInput").ap()
    xT8_d = nc.dram_tensor("xT8", [128, KP, KI, B], fp8, kind="ExternalInput").ap()
    x8b_d = nc.dram_tensor("x8b", [128, MT * D], fp8, kind="ExternalInput").ap()
    eT8_d = nc.dram_tensor("eT8", [128, KP, KI, QS], fp8, kind="ExternalInput").ap()
    osum_d = nc.dram_tensor("osum", [128, MT], f32, kind="ExternalOutput").ap()
    tco_d = nc.dram_tensor("tco", [128, MT], f32, kind="ExternalOutput").ap()

    with tile.TileContext(nc) as tc:
        with (
            tc.tile_pool(name="singles", bufs=1) as singles,
            tc.tile_pool(name="work", bufs=3) as work,
            tc.tile_pool(name="small", bufs=4) as small,
        ):
            ident = singles.tile([128, 128], bf16)
            make_identity(nc, ident)

            # one packed DMA for the small inputs, then big operands
            # interleaved so compute unblocks ASAP
            sm = singles.tile([128, SC], f32)
            nc.sync.dma_start(out=sm, in_=sm_d)
            uniqb = singles.tile([128, U], bf16)
            nc.sync.dma_start(out=uniqb, in_=unq_d)
            rcnt = sm[:, _C_RCNT:_C_RCNT + UT]
            gkr = sm[:, _C_GKR:_C_GKR + UT]
            widx = sm[:, _C_WIDX:_C_WIDX + MT]
            iotab = sm[:, _C_IOTA:_C_IOTA + U]
            pidx = sm[:, _C_PIDX:_C_PIDX + 1]
            labf = sm[:, _C_LABF:_C_LABF + MT]

            BC = B // 4   # b-range per xT8/x8b DMA chunk
            xT8 = singles.tile([128, KP, KI, B], fp8)
            x8b = singles.tile([128, MT, D], fp8)
            mk8 = singles.tile([128, MT, U], fp8)
            eT8 = singles.tile([128, KP, KI, QS], fp8)

            def xT8_chunk(h):
                nc.sync.dma_start(out=xT8[:, :, :, h * BC:(h + 1) * BC],
                                  in_=xT8_d[:, :, :, h * BC:(h + 1) * BC])

            def x8b_chunk(h):
                s = MT // 4 * h
                nc.sync.dma_start(out=x8b[:, s:s + MT // 4, :],
                                  in_=x8b_d[:, s * D:(s + MT // 4) * D])

            x8b_chunk(0)
            x8b_chunk(1)
            xT8_chunk(0)
            x8b_chunk(2)
            x8b_chunk(3)
            xT8_chunk(1)
            xT8_chunk(2)
            xT8_chunk(3)
            for n in range(NQ):
                nc.sync.dma_start(out=eT8[:, :, :, n * 512:(n + 1) * 512],
                                  in_=eT8_d[:, :, :, n * 512:(n + 1) * 512])

            biasM = singles.tile([128, 1], f32)
            nc.vector.memset(biasM, -M)
            # dummy activation at t~0: the single act table (square/copy/exp
            # all co-resident) loads off the critical path
            tinya = singles.tile([128, 1], f32)
            nc.scalar.activation(out=tinya, in_=biasM, func=AF.Square)

            u32 = mybir.dt.uint32

            def emit_rsqrt(dst, src, n, k, eng):
                """dst = k / sqrt(src), elementwise on one engine (no sqrt
                act table): quake-III seed via integer ops + 2 Newton steps."""
                tu = small.tile([128, n], u32, tag=f"qt{n}")
                # seed = bitcast(0x5f3759df - (bits(src) >> 1)), computed
                # wrap-free as (~ (bits >> 1)) - (0xffffffff - MAGIC)
                eng.tensor_scalar(out=tu, in0=src.bitcast(u32),
                                  scalar1=1, scalar2=None,
                                  op0=OP.logical_shift_right)
                eng.tensor_scalar(out=tu, in0=tu,
                                  scalar1=0xFFFFFFFF, scalar2=None,
                                  op0=OP.bitwise_xor)
                eng.tensor_scalar(out=tu, in0=tu,
                                  scalar1=0xFFFFFFFF - 0x5F3759DF,
                                  scalar2=None, op0=OP.subtract)
                r = tu.bitcast(f32)
                a = small.tile([128, n], f32, tag=f"qa{n}")
                for it in range(2):
                    last = it == 1
                    eng.tensor_tensor(out=a, in0=r, in1=r, op=OP.mult)
                    eng.tensor_tensor(out=a, in0=a, in1=src, op=OP.mult)
                    eng.tensor_scalar(out=a, in0=a,
                                      scalar1=-0.5 * (k if last else 1.0),
                                      scalar2=1.5 * (k if last else 1.0),
                                      op0=OP.mult, op1=OP.add)
                    eng.tensor_tensor(out=dst if last else r,
                                      in0=r, in1=a, op=OP.mult)

            # one-hot masks mk8[:, m, u] = (labels[128m+p] == uniq[u]):
            # early tiles on DVE (idle until the diag loop), rest on Pool
            def emit_mask(m, eng):
                eng.tensor_scalar(out=mk8[:, m, :], in0=uniqb,
                                  scalar1=labf[:, m:m + 1], scalar2=None,
                                  op0=OP.is_equal)
            for m in range(16, MT):
                emit_mask(m, nc.gpsimd)
            for m in range(16):
                emit_mask(m, nc.vector)

            uembT8 = singles.tile([128, KP, KI, U], fp8)   # gated fp8 means^T
            nsq = singles.tile([128, MT], f32)             # |x_b|^2
            rin30 = singles.tile([128, MT], f32)           # 30/(SXE*|x_b|)
            macc = singles.tile([128, UT], f32)            # (sum/cnt)^2 norms
            osum = singles.tile([128, MT], f32)            # sum-exp collector
            tco = singles.tile([128, MT], f32)             # target-cos collector

            # ---------- phase A: masked means + row norms, chunk-wise ------
            with (
                tc.tile_pool(name="psum_u", bufs=1, space="PSUM") as psum_u,
                tc.tile_pool(name="psum_n", bufs=5, space="PSUM") as psum_n,
            ):
                ps_u = [psum_u.tile([128, D], f32, tag=f"uniq{mu}",
                                    name=f"ps_u{mu}") for mu in range(UT)]

                def means_pair(t):
                    for mu in range(UT):
                        nc.tensor.matmul(
                            ps_u[mu],
                            mk8[:, 2 * t:2 * t + 2, mu * 128:(mu + 1) * 128],
                            x8b[:, 2 * t:2 * t + 2, :],
                            start=(t == 0), stop=(t == MT // 2 - 1),
                            perf_mode=DR)

                def diag_tile(m):
                    psn = psum_n.tile([128, 512], f32, tag="psn")
                    dv = psn[:, 0:128]
                    xs = xT8[:, :, :, m * 128:(m + 1) * 128]
                    for kp in range(KP):
                        nc.tensor.matmul(dv, xs[:, kp, :, :], xs[:, kp, :, :],
                                         start=(kp == 0), stop=(kp == KP - 1),
                                         perf_mode=DR)
                    scr = work.tile([128, 128], f32, tag="dscr")
                    nc.vector.scalar_tensor_tensor(
                        out=scr, in0=iotab[:, 0:128], scalar=pidx,
                        in1=dv, op0=OP.is_equal, op1=OP.mult,
                        accum_out=nsq[:, m:m + 1])

                # all means first (x8b chunks lead the DMA pipe)
                for t in range(MT // 2):
                    means_pair(t)

                # mean chain: squared mean norms straight from PSUM (ACT),
                # rmg = SXE*gate*ukeep*rcnt/|mean| (rsqrt on DVE, no sqrt
                # table -> the single exp-capable act table loads once at t~0)
                for mu in range(UT):
                    sq2 = work.tile([128, D], bf16, tag="sq")
                    nc.scalar.activation(out=sq2, in_=ps_u[mu],
                                         func=AF.Square,
                                         scale=rcnt[:, mu:mu + 1],
                                         accum_out=macc[:, mu:mu + 1])

                for m in range(0, 8):
                    diag_tile(m)

                mrcq = small.tile([128, UT], f32, tag="mrcq")
                emit_rsqrt(mrcq, macc, UT, 1.0, nc.vector)
                rmg = small.tile([128, UT], f32, tag="rmg")
                nc.vector.tensor_tensor(out=rmg, in0=mrcq, in1=gkr,
                                        op=OP.mult)

                for m in range(8, 24):
                    diag_tile(m)

                # normalized gated means (ACT copy) -> transpose -> fp8 (ACT)
                mng_t = [singles.tile([128, D], bf16, name=f"mng{mu}")
                         for mu in range(UT)]
                for mu in range(UT):
                    nc.scalar.activation(out=mng_t[mu], in_=ps_u[mu],
                                         func=AF.Copy,
                                         scale=rmg[:, mu:mu + 1])
                for mu in range(UT):
                    pst = psum_n.tile([128, D], bf16, tag="pst", bufs=1)
                    for kd in range(D // 128):
                        nc.tensor.transpose(pst[:, kd * 128:(kd + 1) * 128],
                                            mng_t[mu][:, kd * 128:(kd + 1) * 128],
                                            ident)
                    nc.scalar.activation(
                        out=uembT8[:, :, :, mu * 128:(mu + 1) * 128],
                        in_=pst, func=AF.Copy)

                emit_rsqrt(rin30[:, 0:24], nsq[:, 0:24], 24,
                           OIM_SCALAR / SXE, nc.vector)

                for m in range(24, MT):
                    diag_tile(m)

                emit_rsqrt(rin30[:, 24:MT], nsq[:, 24:MT], 8,
                           OIM_SCALAR / SXE, nc.vector)

            # ---------- phase C: logits + exp + sums + target gather -------
            with tc.tile_pool(name="psum_l", bufs=2, space="PSUM") as psum_l:
                for m in range(MT):
                    pl = psum_l.tile([128, NQ * 512], f32, tag="pl")
                    xs = xT8[:, :, :, m * 128:(m + 1) * 128]
                    for n in (1, 2, 3, 0):
                        for kp in range(KP):
                            nc.tensor.matmul(
                                pl[:, n * 512:(n + 1) * 512],
                                xs[:, kp, :, :],
                                eT8[:, kp, :, n * 512:(n + 1) * 512],
                                start=(kp == 0),
                                stop=(kp == KP - 1 and n != 0),
                                perf_mode=DR)
                    # gated means accumulate onto cols 0:U, same group as the
                    # n=0 chunk (eT8 cols 0:U are zero on core 0; uembT8 is
                    # zero on cores 1-7)
                    for kp in range(KP):
                        nc.tensor.matmul(
                            pl[:, 0:U], xs[:, kp, :, :], uembT8[:, kp, :, :],
                            start=False, stop=(kp == KP - 1),
                            perf_mode=DR, skip_group_check=True)
                    # target logit: window cols live at 0:U (core 0 layout)
                    scr = work.tile([128, U], f32, tag="scr")
                    nc.vector.scalar_tensor_tensor(
                        out=scr, in0=iotab, scalar=widx[:, m:m + 1],
                        in1=pl[:, 0:U], op0=OP.is_equal, op1=OP.mult,
                        accum_out=tco[:, m:m + 1])
                    ex = work.tile([128, NQ * 512], bf16, tag="ex", bufs=2)
                    if m == MT - 1:
                        # last tile: ACT accumulator (187ns aux) beats the
                        # DVE round-trip on the drain path
                        nc.scalar.activation(out=ex, in_=pl, func=AF.Exp,
                                             bias=biasM,
                                             scale=rin30[:, m:m + 1],
                                             accum_out=osum[:, m:m + 1])
                    else:
                        nc.scalar.activation(out=ex, in_=pl, func=AF.Exp,
                                             bias=biasM,
                                             scale=rin30[:, m:m + 1])
                        nc.vector.tensor_scalar(out=ex, in0=ex, scalar1=1.0,
                                                scalar2=0.0, op0=OP.mult,
                                                op1=OP.add,
                                                accum_out=osum[:, m:m + 1])
                    if m == MT // 2 - 1:
                        nc.sync.dma_start(out=osum_d[:, 0:MT // 2],
                                          in_=osum[:, 0:MT // 2])
                nc.vector.tensor_tensor(out=tco, in0=tco, in1=rin30,
                                        op=OP.mult)
                nc.sync.dma_start(out=tco_d, in_=tco)

            nc.sync.dma_start(out=osum_d[:, MT // 2:], in_=osum[:, MT // 2:])

    nc.compile()
    return nc


def _host_bookkeeping(labels, label_cq, header_cq):
    """Mirror the reference's integer-only queue-update semantics."""
    labels = np.asarray(labels).astype(np.int64)
    lab = np.asarray(label_cq).astype(np.int64).copy()
    h0 = int(np.asarray(header_cq))

    uq = np.unique(labels)
    if uq.size < U:
        uniq = np.concatenate([uq, np.full(U - uq.size, uq.min(), np.int64)])
    else:
        uniq = uq[:U]
    cnts = np.array([(labels == v).sum() for v in uniq], np.int64)

    emb_src = np.full(Q, -1, np.int64)   # >=0: row u of uniq means; -1: original
    h = h0 % Q
    for u in range(U):
        y = uniq[u]
        m = lab == y
        i = int(np.argmax(m)) if m.any() else 0
        inval = bool(m.any()) and (i != h)
        emb_src[h] = u
        lab[h] = y
        if inval:
            lab[i] = IGNORE
        h = (h + 1) % Q

    good = lab != IGNORE
    goodidx = np.flatnonzero(good)
    gl = lab[goodidx]
    vals, first = np.unique(gl, return_index=True)
    pos = np.searchsorted(vals, labels)
    assert np.all(vals[np.clip(pos, 0, vals.size - 1)] == labels), \
        "batch label missing from queue"
    xe = goodidx[first[pos]]
    return uniq, cnts, emb_src, good, xe


def _pmajor(v, cols):
    return np.ascontiguousarray(np.asarray(v, np.float32).reshape(cols, 128).T)


def _prepare(inputs, labels, emb_cq, label_cq, header_cq):
    inputs = np.asarray(inputs, np.float32)
    emb_cq = np.asarray(emb_cq, np.float32)
    labels = np.asarray(labels)

    uniq, cnts, emb_src, good, xe = _host_bookkeeping(labels, label_cq, header_cq)

    max_nrm = float(np.sqrt((emb_cq.astype(np.float64) ** 2).sum(axis=1).max()))
    M = OIM_SCALAR * max(1.0, max_nrm) * 1.0000001

    window = emb_src >= 0
    u_slot = np.full(U, -1, np.int64)
    wi = np.flatnonzero(window)
    u_slot[emb_src[wi]] = wi
    u_kept = (u_slot >= 0) & good[np.clip(u_slot, 0, Q - 1)]

    w_idx = emb_src[xe].astype(np.float64)        # -1 for non-window targets
    w_idx[w_idx >= 0] = np.where(
        u_kept[w_idx[w_idx >= 0].astype(np.int64)],
        w_idx[w_idx >= 0], -1.0)
    extra = np.flatnonzero(w_idx < 0)             # handled on host (rare/none)

    # ---- device input layouts ----
    x8 = inputs.astype(F8)
    x8b = np.ascontiguousarray(
        x8.reshape(MT, 128, D).transpose(1, 0, 2).reshape(128, MT * D))
    # xT8[p, kp, i, b] = fp8(x[b, 256*kp + 128*i + p])
    xT8 = np.ascontiguousarray(
        x8.T.reshape(KP, KI, 128, B).transpose(2, 0, 1, 3))

    keep_orig = good & ~window
    embq = (SXE * emb_cq).astype(F8)
    embq[~keep_orig] = 0                          # bad or window -> zero cols
    orig_idx = np.flatnonzero(~window)            # Q-U slots, canonical order
    n_orig0 = QS - W0                             # originals on core 0

    nzero = int((~keep_orig[orig_idx]).sum()) + int((~u_kept).sum())

    rcnt = 1.0 / cnts.astype(np.float64)
    gkr_full = SXE * u_kept.astype(np.float64) * rcnt

    def packed_smalls(core0):
        smalls = np.zeros((128, SC), np.float32)
        smalls[:, _C_RCNT:_C_RCNT + UT] = _pmajor(rcnt, UT)
        smalls[:, _C_GKR:_C_GKR + UT] = _pmajor(
            gkr_full if core0 else np.zeros(U), UT)
        smalls[:, _C_WIDX:_C_WIDX + MT] = _pmajor(w_idx, MT)
        smalls[:, _C_IOTA:_C_IOTA + U] = np.arange(U, dtype=np.float32)[None, :]
        smalls[:, _C_PIDX] = np.arange(128, dtype=np.float32)
        smalls[:, _C_LABF:_C_LABF + MT] = _pmajor(
            labels.astype(np.float64), MT)
        return smalls

    base = {
        "xT8": xT8,
        "x8b": x8b,
        "uniqb": np.ascontiguousarray(
            np.broadcast_to(uniq.astype(BF), (128, U))),
    }
    sm0 = packed_smalls(True)
    smn = packed_smalls(False)

    def to_dmajor(cols):
        # cols: [QS, D] fp8 -> [128, KP, KI, QS] with (p,kp,i,j) layout
        t = np.ascontiguousarray(cols).T          # [D, QS]
        return np.ascontiguousarray(
            t.reshape(KP, KI, 128, QS).transpose(2, 0, 1, 3))

    in_maps = []
    for c in range(N_CORES):
        cols = np.zeros((QS, D), F8)
        if c == 0:
            cols[W0:] = embq[orig_idx[:n_orig0]]
        else:
            sl = orig_idx[n_orig0 + (c - 1) * QS: n_orig0 + c * QS]
            cols[:] = embq[sl]
        in_maps.append({**base, "eT8": to_dmajor(cols),
                        "smalls": sm0 if c == 0 else smn})
    return M, in_maps, extra, xe, nzero


def _combine(res_list, M, extra, xe, nzero, inputs, emb_cq):
    S = np.zeros(B, np.float64)
    for r in res_list:
        S += r["osum"].astype(np.float64).T.reshape(B)
    S -= nzero * np.exp(-np.float64(M))
    t30 = res_list[0]["tco"].astype(np.float64).T.reshape(B)

    if extra.size:  # targets pointing at original (non-window) queue rows
        xb = np.asarray(inputs, np.float64)[extra]
        xb /= np.maximum(np.linalg.norm(xb, axis=1, keepdims=True), 1e-12)
        eb = np.asarray(emb_cq, np.float64)[xe[extra]]
        t30[extra] = OIM_SCALAR * (xb * eb).sum(axis=1)

    loss = np.mean(M + np.log(S) - t30)
    return np.array(loss, dtype=np.float32)


def kernel(inputs, labels, emb_cq, label_cq, age_cq, header_cq):
    from concourse.bass_utils import run_bass_kernel_spmd

    M, in_maps, extra, xe, nzero = _prepare(
        inputs, labels, emb_cq, label_cq, header_cq)

    key = round(M, 9)
    if key not in _PROG_CACHE:
        _PROG_CACHE[key] = _build_program(M)
    nc = _PROG_CACHE[key]

    res = run_bass_kernel_spmd(nc, in_maps, core_ids=list(range(N_CORES)))
    return _combine(res.results, M, extra, xe, nzero, inputs, emb_cq)
